# revision 26
# baseline (speedup 1.0000x reference)
"""RNN-T transducer loss on TRN2 — lag-2 skewed-wavefront blocked-scan kernel.

8 cores run 8 independent DP chains (4 sequences x {fwd rows u=1..48,
bwd rows v=1..47 reversed-coords, padded}).  Each chain's 48x512
lattice block: t axis cut into C=8 chunks of L=64; one
tensor_tensor_scan per schedule step processes cells (u, c) with
u = s - 2c on C contiguous partition lanes.  TRN2 forbids +-1
partition moves on compute engines (32-aligned bases, contiguous
windows, shared input bases), so inter-chunk carries go through the
PE: a superdiagonal [C,C] matmul shifts the carry column into PSUM,
which the scan's `initial` operand reads (PSUM is exempt from the
SBUF same-base rule).  The lag-2 skew (cell (u,c) at step u+2c) gives
the PE round-trip two steps of slack, keeping the DVE critical path
pure scans.  Inactive lanes get d0=0 coefficients: the scan
degenerates to a copy, which parks finished row-48 chunks and carries
the init row forward, so the final buffer holds the full seam row.

Transform: W_u[t] = exp(alpha[t,u] - S0[t] - E_u[t]) with E_u the
cross-row emit cumsum and S0 = alpha[t,0] + g(t), g a fitted
sqrt-envelope profile.  Cross-row scan coefficient is exactly 1, all
intermediates stay in fp32 range, and cells far below the envelope
underflow to 0 harmlessly — no mid-lattice rescaling.  Host does the
O(T*U) packing and the f64 seam combine; the device executes every
lattice cell update.
"""
import numpy as np

B, T, U, D = 4, 512, 97, 512
NR = 48                      # rows per chain (bwd pads its 48th row with zeros)
C = 8                        # t-chunks (contiguous scan lanes)
L = T // C                   # elements per chunk
S = NR + 2 * (C - 1)         # schedule steps (lag-2 skew)
NDMA = 2                     # coefficient DMA splits (issued from SP + ACT)
HSHIFT = 25.0                # downward shift of the envelope profile


def _g_profile():
    t = np.arange(T, dtype=np.float64)
    return 17.22 * np.sqrt(t) - 0.092 * t - 1.94 - HSHIFT


def _install_shims():
    import sys, types
    try:
        import antenv.axon_hooks  # noqa: F401
    except Exception:
        m = types.ModuleType("antenv.axon_hooks")
        m._hook = None
        m.set_axon_ntff_profile_hook = lambda h: setattr(m, "_hook", h)
        m.get_axon_ntff_profile_hook = lambda: getattr(m, "_hook", None)
        sys.modules["antenv.axon_hooks"] = m
        try:
            import antenv
            antenv.axon_hooks = m
        except Exception:
            pass
        try:
            from trn_agent_boot.trn_boot import _ntff_profile_via_ctypes
            hk = _ntff_profile_via_ctypes("/opt/axon/libaxon_pjrt.so")
            if hk is not None:
                m.set_axon_ntff_profile_hook(hk)
        except Exception:
            pass

    # Split the TileContext final-drain sem waits across multiple drain
    # instructions: the CTRL encoding holds too few wait slots and the
    # walrus backend rejects the fused drain ("Too many sync wait commands").
    import concourse.tile as _tile
    from concourse import mybir as _mybir
    from concourse.vector_clock import ScopedClock as _ScopedClock

    if getattr(_tile.TileContext, "_drain_patched", False):
        return

    def _patched_drain_and_barrier(self, tick_clock, wait_clock):
        nc = self.nc
        drain_inst = nc.sync.drain()
        wait_clock.add_sem_waits(
            drain_inst.ins, _ScopedClock({None: tick_clock.global_clock})
        )
        si = drain_inst.ins.sync_info
        waits = list(si.on_wait) if si is not None else []
        if len(waits) > 1:
            ups = list(si.on_update) if si is not None else []
            drain_inst.ins.sync_info = _mybir.SyncInfo(on_wait=waits[:1], on_update=ups)
            for i in range(1, len(waits)):
                extra = nc.sync.drain()
                extra.ins.sync_info = _mybir.SyncInfo(
                    on_wait=waits[i : i + 1], on_update=[]
                )
        nc.all_engine_barrier()
        assert self.sems is not None
        popped = nc._tile_sem_poison_stack.pop()
        assert popped is self._sem_poison
        nc.clear_and_free_semaphores(list(self.sems.allocated().values()))
        nc.all_engine_barrier()

    _tile.TileContext._drain_and_barrier = _patched_drain_and_barrier
    _tile.TileContext._drain_patched = True


def _build_nc():
    from concourse import bass, mybir

    f32 = mybir.dt.float32
    bf16 = mybir.dt.bfloat16
    nc = bass.Bass()
    SL = S * L
    W = SL // NDMA
    cop = [
        nc.declare_dram_parameter(f"cop{i}", [C, W], bf16, isOutput=False)
        for i in range(NDMA)
    ]
    v0p = nc.declare_dram_parameter("v0", [C, L], bf16, isOutput=False)
    shp = nc.declare_dram_parameter("sh", [C, C], bf16, isOutput=False)
    outp = nc.declare_dram_parameter("outW", [C, L], bf16, isOutput=True)

    co = nc.alloc_sbuf_tensor("co", [C, SL], bf16)
    b0 = nc.alloc_sbuf_tensor("b0", [C, L], bf16)
    b1 = nc.alloc_sbuf_tensor("b1", [C, L], bf16)
    sht = nc.alloc_sbuf_tensor("sht", [C, C], bf16)
    pc0 = nc.alloc_psum_tensor("pc0", [C, 1], f32)
    pc1 = nc.alloc_psum_tensor("pc1", [C, 1], f32)

    dma_sem = nc.alloc_semaphore("dma_sem")
    step_sem = nc.alloc_semaphore("step_sem")
    out_sem = nc.alloc_semaphore("out_sem")

    bufs = [b0, b1]
    pcs = [pc0, pc1]
    TOTAL = 2 + S + (S - 2)

    with nc.Block("main", no_gpsimd_drain=True) as blk:

        @blk.sync
        def _(sync):
            sync.dma_start(out=co[:, 0:W], in_=cop[0][:]).then_inc(dma_sem, 16)
            sync.dma_start(out=b0[:], in_=v0p[:]).then_inc(dma_sem, 16)
            sync.wait_ge(step_sem, TOTAL)
            sync.dma_start(out=outp[:], in_=bufs[S % 2][:]).then_inc(out_sem, 16)
            sync.wait_ge(out_sem, 16)

        @blk.scalar
        def _(scalar):
            scalar.dma_start(out=co[:, W : 2 * W], in_=cop[1][:]).then_inc(dma_sem, 16)
            scalar.dma_start(out=sht[:], in_=shp[:]).then_inc(dma_sem, 16)

        @blk.tensor
        def _(tensor):
            tensor.wait_ge(dma_sem, 64)
            nc.tensor.matmul(
                pc0[:, 0:1], sht[:], sht[:, 0:1], start=True, stop=True
            ).then_inc(step_sem, 1)
            nc.tensor.matmul(
                pc1[:, 0:1], sht[:], sht[:, 0:1], start=True, stop=True
            ).then_inc(step_sem, 1)
            for s in range(1, S - 1):
                tensor.wait_ge(step_sem, 2 * s + 1)
                nc.tensor.matmul(
                    pcs[s % 2][:, 0:1], sht[:], bufs[s % 2][:, L - 1 : L],
                    start=True, stop=True,
                ).then_inc(step_sem, 1)

        @blk.vector
        def _(vector):
            for s in range(1, S + 1):
                vector.wait_ge(step_sem, 2 + (s - 1) + max(0, s - 2))
                nc.vector.tensor_tensor_scan(
                    out=bufs[s % 2][:],
                    data0=co[:, (s - 1) * L : s * L],
                    data1=bufs[(s - 1) % 2][:],
                    initial=pcs[s % 2][:, 0:1],
                    op0=mybir.AluOpType.mult,
                    op1=mybir.AluOpType.add,
                ).then_inc(step_sem, 1)

    nc.finalize()
    _check_single_waits(nc)
    return nc


def _check_single_waits(nc):
    f = nc.m.functions[0]
    for attr in ("basic_blocks", "bbs", "blocks"):
        if hasattr(f, attr):
            bbs = getattr(f, attr)
            break
    else:
        return
    bad = []
    for bb in bbs:
        for ins in bb.instructions:
            si = ins.sync_info
            if si is not None and len(si.on_wait) > 1 and ins.opcode in (
                "TensorScalarPtr", "Matmult"
            ):
                bad.append((ins.name, ins.opcode, len(si.on_wait)))
    assert not bad, f"multi-wait hot instructions: {bad[:5]}"


def _shift_matrix():
    import ml_dtypes
    sh = np.zeros((C, C), ml_dtypes.bfloat16)
    for c in range(1, C):
        sh[c - 1, c] = 1.0          # out[c] = carry[c-1]
    return sh


def _pack_chain(lbx, lex, nrows):
    """lbx [T, nrows+1] blank col per row (col 0 drives S0), lex [T, >=nrows]
    emit cols (row u uses col u-1).  Returns (coef [C, S*L] f32,
    v0 [C, L] f32, S0 [T] f64, Efinal [T] f64 = E_{nrows}[t])."""
    g = _g_profile()
    S0 = np.zeros(T)
    S0[1:] = np.cumsum(lbx[:-1, 0])
    S0 += g
    E = np.zeros((T, nrows + 1))
    E[:, 1:] = np.cumsum(lex[:, :nrows], axis=1)
    # d0_u[t] = exp(lbx[t-1,u] + S0[t-1]-S0[t] + E_u[t-1]-E_u[t]);  d0_u[0]=0
    d0 = np.zeros((NR + 1, T), np.float32)
    uu = np.arange(1, nrows + 1)
    ld = lbx[:-1, uu] + (S0[:-1] - S0[1:])[:, None] + E[:-1, uu] - E[1:, uu]
    d0[1 : nrows + 1, 1:] = np.exp(ld).T.astype(np.float32)
    v0 = np.exp(-g).astype(np.float32).reshape(C, L)
    coef = np.zeros((C, S * L), np.float32)
    for s in range(1, S + 1):
        for c in range(C):
            u = s - 2 * c
            if 1 <= u <= NR:
                coef[c, (s - 1) * L : s * L] = d0[u, c * L : (c + 1) * L]
    return coef, v0, S0, E[:, nrows]


def _sim_chain(coef, v0):
    """Numpy simulation of the device schedule (fp32), for validation."""
    bufs = [v0.astype(np.float32).copy(), np.zeros((C, L), np.float32)]
    carr = [np.zeros(C, np.float32), np.zeros(C, np.float32)]
    for s in range(1, S + 1):
        cur = bufs[(s - 1) % 2]
        cf = coef[:, (s - 1) * L : s * L]
        state = carr[s % 2].copy()
        out = np.empty((C, L), np.float32)
        for j in range(L):
            state = cf[:, j] * state + cur[:, j]
            out[:, j] = state
        bufs[s % 2][:] = out
        if s <= S - 2:
            carr[s % 2][1:] = out[0 : C - 1, L - 1]
            carr[s % 2][0] = 0.0
    return bufs[S % 2].reshape(-1)


_RUN_STATE = {}


def _prep(inputs):
    lp = np.asarray(inputs["log_probs"], dtype=np.float32)
    tgt = np.asarray(inputs["targets"]).astype(np.int64)
    blank = int(inputs["blank"])
    lb = lp[:, :, :, blank].astype(np.float64)                     # [B,T,U]
    le = np.take_along_axis(
        lp[:, :, : U - 1, :], tgt[:, None, :, None], axis=3
    )[..., 0].astype(np.float64)                                   # [B,T,U-1]

    in_maps, recon = [], []
    sh = _shift_matrix()
    W = S * L // NDMA
    s_ = np.arange(T - 1)
    tau = np.arange(T)

    import ml_dtypes
    bf16 = ml_dtypes.bfloat16

    def add_map(coef, v0):
        m = {f"cop{i}": np.ascontiguousarray(coef[:, i * W : (i + 1) * W]).astype(bf16)
             for i in range(NDMA)}
        m["v0"] = v0.astype(bf16)
        m["sh"] = sh
        in_maps.append(m)

    for b in range(B):
        coef, v0, S0, Ef = _pack_chain(lb[b, :, : NR + 1], le[b, :, :NR], NR)
        add_map(coef, v0)
        recon.append((S0, Ef))
    for b in range(B):
        lbr = np.zeros((T, NR + 1))
        for v in range(NR + 1):
            lbr[:-1, v] = lb[b, T - 2 - s_, U - 1 - v]
        ler = np.zeros((T, NR))
        for w in range(NR - 1):
            ler[:, w] = le[b, T - 1 - tau, U - 2 - w]
        coef, v0, S0, Er = _pack_chain(lbr, ler, NR - 1)
        add_map(coef, v0)
        recon.append((S0, Er))
    return lb, le, in_maps, recon


def _combine(lb, le, recon, Wf_all):
    tau = np.arange(T)
    costs = np.empty(B, np.float32)
    for b in range(B):
        Wf = Wf_all[b].astype(np.float64)
        Wr = Wf_all[4 + b].astype(np.float64)
        S0f, Ef = recon[b]
        S0r, Er = recon[4 + b]
        with np.errstate(divide="ignore"):
            alphaf = np.log(Wf) + S0f + Ef                          # alpha[t, 48]
            base = lb[b, T - 1, U - 1]
            betar = np.log(Wr) + base + S0r + Er                    # beta-hat[tau, 47]
        beta49 = betar[T - 1 - tau]                                  # beta[t, 49]
        terms = alphaf + le[b, :, NR] + beta49
        mx = terms.max()
        costs[b] = np.float32(-(mx + np.log(np.sum(np.exp(terms - mx)))))
    return costs


def kernel(**inputs) -> np.ndarray:
    _install_shims()
    from concourse.bass_utils import run_bass_kernel_spmd

    lb, le, in_maps, recon = _prep(inputs)
    nc = _build_nc()
    r = run_bass_kernel_spmd(
        nc, in_maps, list(range(8)), trace=_RUN_STATE.get("trace", False)
    )
    _RUN_STATE["last"] = r
    Wf_all = [r.results[i]["outW"].astype(np.float32).reshape(-1) for i in range(8)]
    return _combine(lb, le, recon, Wf_all)


# revision 27
# speedup vs baseline: 1.1619x; 1.1619x over previous
"""RNN-T transducer loss on TRN2 — lag-2 skewed-wavefront blocked-scan kernel.

8 cores run 8 independent DP chains (4 sequences x {fwd rows u=1..48,
bwd rows v=1..47 reversed-coords, padded}).  Each chain's 48x512
lattice block: t axis cut into C=8 chunks of L=64; one
tensor_tensor_scan per schedule step processes cells (u, c) with
u = s - 2c on C contiguous partition lanes.  TRN2 forbids +-1
partition moves on compute engines (32-aligned bases, contiguous
windows, shared input bases), so inter-chunk carries go through the
PE: a superdiagonal [C,C] matmul shifts the carry column into PSUM,
which the scan's `initial` operand reads (PSUM is exempt from the
SBUF same-base rule).  The lag-2 skew (cell (u,c) at step u+2c) gives
the PE round-trip two steps of slack, keeping the DVE critical path
pure scans.  Inactive lanes get d0=0 coefficients: the scan
degenerates to a copy, which parks finished row-48 chunks and carries
the init row forward, so the final buffer holds the full seam row.

Transform: W_u[t] = exp(alpha[t,u] - S0[t] - E_u[t]) with E_u the
cross-row emit cumsum and S0 = alpha[t,0] + g(t), g a fitted
sqrt-envelope profile.  Cross-row scan coefficient is exactly 1, all
intermediates stay in fp32 range, and cells far below the envelope
underflow to 0 harmlessly — no mid-lattice rescaling.  Host does the
O(T*U) packing and the f64 seam combine; the device executes every
lattice cell update.
"""
import numpy as np

B, T, U, D = 4, 512, 97, 512
NR = 48                      # rows per chain (bwd pads its 48th row with zeros)
C = 8                        # t-chunks (contiguous scan lanes)
L = T // C                   # elements per chunk
S = NR + 2 * (C - 1)         # schedule steps (lag-2 skew)
NDMA = 2                     # coefficient DMA splits (issued from SP + ACT)
HSHIFT = 25.0                # downward shift of the envelope profile


def _g_profile():
    t = np.arange(T, dtype=np.float64)
    return 17.22 * np.sqrt(t) - 0.092 * t - 1.94 - HSHIFT


def _install_shims():
    import sys, types
    try:
        import antenv.axon_hooks  # noqa: F401
    except Exception:
        m = types.ModuleType("antenv.axon_hooks")
        m._hook = None
        m.set_axon_ntff_profile_hook = lambda h: setattr(m, "_hook", h)
        m.get_axon_ntff_profile_hook = lambda: getattr(m, "_hook", None)
        sys.modules["antenv.axon_hooks"] = m
        try:
            import antenv
            antenv.axon_hooks = m
        except Exception:
            pass
        try:
            from trn_agent_boot.trn_boot import _ntff_profile_via_ctypes
            hk = _ntff_profile_via_ctypes("/opt/axon/libaxon_pjrt.so")
            if hk is not None:
                m.set_axon_ntff_profile_hook(hk)
        except Exception:
            pass

    # Split the TileContext final-drain sem waits across multiple drain
    # instructions: the CTRL encoding holds too few wait slots and the
    # walrus backend rejects the fused drain ("Too many sync wait commands").
    import concourse.tile as _tile
    from concourse import mybir as _mybir
    from concourse.vector_clock import ScopedClock as _ScopedClock

    if getattr(_tile.TileContext, "_drain_patched", False):
        return

    def _patched_drain_and_barrier(self, tick_clock, wait_clock):
        nc = self.nc
        drain_inst = nc.sync.drain()
        wait_clock.add_sem_waits(
            drain_inst.ins, _ScopedClock({None: tick_clock.global_clock})
        )
        si = drain_inst.ins.sync_info
        waits = list(si.on_wait) if si is not None else []
        if len(waits) > 1:
            ups = list(si.on_update) if si is not None else []
            drain_inst.ins.sync_info = _mybir.SyncInfo(on_wait=waits[:1], on_update=ups)
            for i in range(1, len(waits)):
                extra = nc.sync.drain()
                extra.ins.sync_info = _mybir.SyncInfo(
                    on_wait=waits[i : i + 1], on_update=[]
                )
        nc.all_engine_barrier()
        assert self.sems is not None
        popped = nc._tile_sem_poison_stack.pop()
        assert popped is self._sem_poison
        nc.clear_and_free_semaphores(list(self.sems.allocated().values()))
        nc.all_engine_barrier()

    _tile.TileContext._drain_and_barrier = _patched_drain_and_barrier
    _tile.TileContext._drain_patched = True


def _build_nc():
    from concourse import bass, mybir

    f32 = mybir.dt.float32
    bf16 = mybir.dt.bfloat16
    nc = bass.Bass()
    SL = S * L
    W = SL // NDMA
    cop = [
        nc.declare_dram_parameter(f"cop{i}", [C, W], bf16, isOutput=False)
        for i in range(NDMA)
    ]
    v0p = nc.declare_dram_parameter("v0", [C, L], bf16, isOutput=False)
    shp = nc.declare_dram_parameter("sh", [C, C], bf16, isOutput=False)
    outp = nc.declare_dram_parameter("outW", [C, L], bf16, isOutput=True)

    co = nc.alloc_sbuf_tensor("co", [C, SL], bf16)
    b0 = nc.alloc_sbuf_tensor("b0", [C, L], bf16)
    b1 = nc.alloc_sbuf_tensor("b1", [C, L], bf16)
    sht = nc.alloc_sbuf_tensor("sht", [C, C], bf16)
    pc0 = nc.alloc_psum_tensor("pc0", [C, 1], f32)
    pc1 = nc.alloc_psum_tensor("pc1", [C, 1], f32)

    dma_sem = nc.alloc_semaphore("dma_sem")
    step_sem = nc.alloc_semaphore("step_sem")
    out_sem = nc.alloc_semaphore("out_sem")

    bufs = [b0, b1]
    pcs = [pc0, pc1]
    TOTAL = 2 + S + (S - 2)

    with nc.Block("main", no_gpsimd_drain=True) as blk:

        @blk.sync
        def _(sync):
            sync.dma_start(out=co[:, 0:W], in_=cop[0][:]).then_inc(dma_sem, 16)
            sync.dma_start(out=b0[:], in_=v0p[:]).then_inc(dma_sem, 16)
            sync.wait_ge(step_sem, TOTAL)
            sync.dma_start(out=outp[:], in_=bufs[S % 2][:]).then_inc(out_sem, 16)
            sync.wait_ge(out_sem, 16)

        @blk.scalar
        def _(scalar):
            scalar.dma_start(out=co[:, W : 2 * W], in_=cop[1][:]).then_inc(dma_sem, 16)
            scalar.dma_start(out=sht[:], in_=shp[:]).then_inc(dma_sem, 16)

        @blk.tensor
        def _(tensor):
            tensor.wait_ge(dma_sem, 64)
            nc.tensor.matmul(
                pc0[:, 0:1], sht[:], sht[:, 0:1], start=True, stop=True
            ).then_inc(step_sem, 1)
            nc.tensor.matmul(
                pc1[:, 0:1], sht[:], sht[:, 0:1], start=True, stop=True
            ).then_inc(step_sem, 1)
            for s in range(1, S - 1):
                tensor.wait_ge(step_sem, 2 * s + 1)
                nc.tensor.matmul(
                    pcs[s % 2][:, 0:1], sht[:], bufs[s % 2][:, L - 1 : L],
                    start=True, stop=True,
                ).then_inc(step_sem, 1)

        @blk.vector
        def _(vector):
            for s in range(1, S + 1):
                vector.wait_ge(step_sem, 2 + (s - 1) + max(0, s - 2))
                nc.vector.tensor_tensor_scan(
                    out=bufs[s % 2][:],
                    data0=co[:, (s - 1) * L : s * L],
                    data1=bufs[(s - 1) % 2][:],
                    initial=pcs[s % 2][:, 0:1],
                    op0=mybir.AluOpType.mult,
                    op1=mybir.AluOpType.add,
                ).then_inc(step_sem, 1)

    _fuse_event_waits(nc, mybir)
    nc.finalize()
    _check_single_waits(nc)
    return nc


def _fuse_event_waits(nc, mybir):
    """Merge standalone wait_ge EventSemaphore instructions into the next
    same-engine instruction's sync_info (saves ~80ns/step on the DVE)."""
    f = nc.m.functions[0]
    for attr in ("basic_blocks", "bbs", "blocks"):
        if hasattr(f, attr):
            bbs = getattr(f, attr)
            break
    else:
        return
    for bb in bbs:
        insts = list(bb.instructions)
        drop = set()
        pending = {}
        for ins in insts:
            eng = getattr(ins, "engine", None)
            si = ins.sync_info
            if (
                ins.opcode == "EventSemaphore"
                and si is not None
                and len(si.on_wait) == 1
                and len(si.on_update) == 0
                and eng in (mybir.EngineType.DVE, mybir.EngineType.PE,
                            mybir.EngineType.SP)
            ):
                pending.setdefault(eng, []).append(ins)
                continue
            if eng in pending and pending[eng]:
                if ins.opcode in ("TensorScalarPtr", "Matmult", "DMACopy"):
                    waits = list(si.on_wait) if si else []
                    ups = list(si.on_update) if si else []
                    for ev in pending[eng]:
                        waits += list(ev.sync_info.on_wait)
                        drop.add(id(ev))
                    if len(waits) <= 1:
                        ins.sync_info = mybir.SyncInfo(
                            on_wait=waits, on_update=ups
                        )
                    else:
                        # cannot fuse safely; keep events standalone
                        for ev in pending[eng]:
                            drop.discard(id(ev))
                pending[eng] = []
        if drop:
            bb.instructions = [i for i in insts if id(i) not in drop]


def _check_single_waits(nc):
    f = nc.m.functions[0]
    for attr in ("basic_blocks", "bbs", "blocks"):
        if hasattr(f, attr):
            bbs = getattr(f, attr)
            break
    else:
        return
    bad = []
    for bb in bbs:
        for ins in bb.instructions:
            si = ins.sync_info
            if si is not None and len(si.on_wait) > 1 and ins.opcode in (
                "TensorScalarPtr", "Matmult"
            ):
                bad.append((ins.name, ins.opcode, len(si.on_wait)))
    assert not bad, f"multi-wait hot instructions: {bad[:5]}"


def _shift_matrix():
    import ml_dtypes
    sh = np.zeros((C, C), ml_dtypes.bfloat16)
    for c in range(1, C):
        sh[c - 1, c] = 1.0          # out[c] = carry[c-1]
    return sh


def _pack_chain(lbx, lex, nrows):
    """lbx [T, nrows+1] blank col per row (col 0 drives S0), lex [T, >=nrows]
    emit cols (row u uses col u-1).  Returns (coef [C, S*L] f32,
    v0 [C, L] f32, S0 [T] f64, Efinal [T] f64 = E_{nrows}[t])."""
    g = _g_profile()
    S0 = np.zeros(T)
    S0[1:] = np.cumsum(lbx[:-1, 0])
    S0 += g
    E = np.zeros((T, nrows + 1))
    E[:, 1:] = np.cumsum(lex[:, :nrows], axis=1)
    # d0_u[t] = exp(lbx[t-1,u] + S0[t-1]-S0[t] + E_u[t-1]-E_u[t]);  d0_u[0]=0
    d0 = np.zeros((NR + 1, T), np.float32)
    uu = np.arange(1, nrows + 1)
    ld = lbx[:-1, uu] + (S0[:-1] - S0[1:])[:, None] + E[:-1, uu] - E[1:, uu]
    d0[1 : nrows + 1, 1:] = np.exp(ld).T.astype(np.float32)
    v0 = np.exp(-g).astype(np.float32).reshape(C, L)
    coef = np.zeros((C, S * L), np.float32)
    for s in range(1, S + 1):
        for c in range(C):
            u = s - 2 * c
            if 1 <= u <= NR:
                coef[c, (s - 1) * L : s * L] = d0[u, c * L : (c + 1) * L]
    return coef, v0, S0, E[:, nrows]


def _sim_chain(coef, v0):
    """Numpy simulation of the device schedule (fp32), for validation."""
    bufs = [v0.astype(np.float32).copy(), np.zeros((C, L), np.float32)]
    carr = [np.zeros(C, np.float32), np.zeros(C, np.float32)]
    for s in range(1, S + 1):
        cur = bufs[(s - 1) % 2]
        cf = coef[:, (s - 1) * L : s * L]
        state = carr[s % 2].copy()
        out = np.empty((C, L), np.float32)
        for j in range(L):
            state = cf[:, j] * state + cur[:, j]
            out[:, j] = state
        bufs[s % 2][:] = out
        if s <= S - 2:
            carr[s % 2][1:] = out[0 : C - 1, L - 1]
            carr[s % 2][0] = 0.0
    return bufs[S % 2].reshape(-1)


_RUN_STATE = {}


def _prep(inputs):
    lp = np.asarray(inputs["log_probs"], dtype=np.float32)
    tgt = np.asarray(inputs["targets"]).astype(np.int64)
    blank = int(inputs["blank"])
    lb = lp[:, :, :, blank].astype(np.float64)                     # [B,T,U]
    le = np.take_along_axis(
        lp[:, :, : U - 1, :], tgt[:, None, :, None], axis=3
    )[..., 0].astype(np.float64)                                   # [B,T,U-1]

    in_maps, recon = [], []
    sh = _shift_matrix()
    W = S * L // NDMA
    s_ = np.arange(T - 1)
    tau = np.arange(T)

    import ml_dtypes
    bf16 = ml_dtypes.bfloat16

    def add_map(coef, v0):
        m = {f"cop{i}": np.ascontiguousarray(coef[:, i * W : (i + 1) * W]).astype(bf16)
             for i in range(NDMA)}
        m["v0"] = v0.astype(bf16)
        m["sh"] = sh
        in_maps.append(m)

    for b in range(B):
        coef, v0, S0, Ef = _pack_chain(lb[b, :, : NR + 1], le[b, :, :NR], NR)
        add_map(coef, v0)
        recon.append((S0, Ef))
    for b in range(B):
        lbr = np.zeros((T, NR + 1))
        for v in range(NR + 1):
            lbr[:-1, v] = lb[b, T - 2 - s_, U - 1 - v]
        ler = np.zeros((T, NR))
        for w in range(NR - 1):
            ler[:, w] = le[b, T - 1 - tau, U - 2 - w]
        coef, v0, S0, Er = _pack_chain(lbr, ler, NR - 1)
        add_map(coef, v0)
        recon.append((S0, Er))
    return lb, le, in_maps, recon


def _combine(lb, le, recon, Wf_all):
    tau = np.arange(T)
    costs = np.empty(B, np.float32)
    for b in range(B):
        Wf = Wf_all[b].astype(np.float64)
        Wr = Wf_all[4 + b].astype(np.float64)
        S0f, Ef = recon[b]
        S0r, Er = recon[4 + b]
        with np.errstate(divide="ignore"):
            alphaf = np.log(Wf) + S0f + Ef                          # alpha[t, 48]
            base = lb[b, T - 1, U - 1]
            betar = np.log(Wr) + base + S0r + Er                    # beta-hat[tau, 47]
        beta49 = betar[T - 1 - tau]                                  # beta[t, 49]
        terms = alphaf + le[b, :, NR] + beta49
        mx = terms.max()
        costs[b] = np.float32(-(mx + np.log(np.sum(np.exp(terms - mx)))))
    return costs


def kernel(**inputs) -> np.ndarray:
    _install_shims()
    from concourse.bass_utils import run_bass_kernel_spmd

    lb, le, in_maps, recon = _prep(inputs)
    nc = _build_nc()
    r = run_bass_kernel_spmd(
        nc, in_maps, list(range(8)), trace=_RUN_STATE.get("trace", False)
    )
    _RUN_STATE["last"] = r
    Wf_all = [r.results[i]["outW"].astype(np.float32).reshape(-1) for i in range(8)]
    return _combine(lb, le, recon, Wf_all)


# revision 28
# speedup vs baseline: 1.1703x; 1.0072x over previous
"""RNN-T transducer loss on TRN2 — lag-2 skewed-wavefront blocked-scan kernel.

8 cores run 8 independent DP chains (4 sequences x {fwd rows u=1..48,
bwd rows v=1..47 reversed-coords, padded}).  Each chain's 48x512
lattice block: t axis cut into C=8 chunks of L=64; one
tensor_tensor_scan per schedule step processes cells (u, c) with
u = s - 2c on C contiguous partition lanes.  TRN2 forbids +-1
partition moves on compute engines (32-aligned bases, contiguous
windows, shared input bases), so inter-chunk carries go through the
PE: a superdiagonal [C,C] matmul shifts the carry column into PSUM,
which the scan's `initial` operand reads (PSUM is exempt from the
SBUF same-base rule).  The lag-2 skew (cell (u,c) at step u+2c) gives
the PE round-trip two steps of slack, keeping the DVE critical path
pure scans.  Inactive lanes get d0=0 coefficients: the scan
degenerates to a copy, which parks finished row-48 chunks and carries
the init row forward, so the final buffer holds the full seam row.

Transform: W_u[t] = exp(alpha[t,u] - S0[t] - E_u[t]) with E_u the
cross-row emit cumsum and S0 = alpha[t,0] + g(t), g a fitted
sqrt-envelope profile.  Cross-row scan coefficient is exactly 1, all
intermediates stay in fp32 range, and cells far below the envelope
underflow to 0 harmlessly — no mid-lattice rescaling.  Host does the
O(T*U) packing and the f64 seam combine; the device executes every
lattice cell update.
"""
import numpy as np

B, T, U, D = 4, 512, 97, 512
NR = 48                      # rows per chain (bwd pads its 48th row with zeros)
C = 8                        # t-chunks (contiguous scan lanes)
L = T // C                   # elements per chunk
S = NR + 2 * (C - 1)         # schedule steps (lag-2 skew)
NDMA = 2                     # coefficient DMA splits (issued from SP + ACT)
SHEAD = 16                   # steps covered by the head coefficient DMA
HSHIFT = 25.0                # downward shift of the envelope profile


def _g_profile():
    t = np.arange(T, dtype=np.float64)
    return 17.22 * np.sqrt(t) - 0.092 * t - 1.94 - HSHIFT


def _install_shims():
    import sys, types
    try:
        import antenv.axon_hooks  # noqa: F401
    except Exception:
        m = types.ModuleType("antenv.axon_hooks")
        m._hook = None
        m.set_axon_ntff_profile_hook = lambda h: setattr(m, "_hook", h)
        m.get_axon_ntff_profile_hook = lambda: getattr(m, "_hook", None)
        sys.modules["antenv.axon_hooks"] = m
        try:
            import antenv
            antenv.axon_hooks = m
        except Exception:
            pass
        try:
            from trn_agent_boot.trn_boot import _ntff_profile_via_ctypes
            hk = _ntff_profile_via_ctypes("/opt/axon/libaxon_pjrt.so")
            if hk is not None:
                m.set_axon_ntff_profile_hook(hk)
        except Exception:
            pass

    # Split the TileContext final-drain sem waits across multiple drain
    # instructions: the CTRL encoding holds too few wait slots and the
    # walrus backend rejects the fused drain ("Too many sync wait commands").
    import concourse.tile as _tile
    from concourse import mybir as _mybir
    from concourse.vector_clock import ScopedClock as _ScopedClock

    if getattr(_tile.TileContext, "_drain_patched", False):
        return

    def _patched_drain_and_barrier(self, tick_clock, wait_clock):
        nc = self.nc
        drain_inst = nc.sync.drain()
        wait_clock.add_sem_waits(
            drain_inst.ins, _ScopedClock({None: tick_clock.global_clock})
        )
        si = drain_inst.ins.sync_info
        waits = list(si.on_wait) if si is not None else []
        if len(waits) > 1:
            ups = list(si.on_update) if si is not None else []
            drain_inst.ins.sync_info = _mybir.SyncInfo(on_wait=waits[:1], on_update=ups)
            for i in range(1, len(waits)):
                extra = nc.sync.drain()
                extra.ins.sync_info = _mybir.SyncInfo(
                    on_wait=waits[i : i + 1], on_update=[]
                )
        nc.all_engine_barrier()
        assert self.sems is not None
        popped = nc._tile_sem_poison_stack.pop()
        assert popped is self._sem_poison
        nc.clear_and_free_semaphores(list(self.sems.allocated().values()))
        nc.all_engine_barrier()

    _tile.TileContext._drain_and_barrier = _patched_drain_and_barrier
    _tile.TileContext._drain_patched = True


def _build_nc():
    from concourse import bass, mybir

    f32 = mybir.dt.float32
    bf16 = mybir.dt.bfloat16
    nc = bass.Bass()
    SL = S * L
    W = SL // NDMA
    cop = [
        nc.declare_dram_parameter("cop0", [C, SHEAD * L], bf16, isOutput=False),
        nc.declare_dram_parameter("cop1", [C, (S - SHEAD) * L], bf16, isOutput=False),
    ]
    v0p = nc.declare_dram_parameter("v0", [C, L], bf16, isOutput=False)
    shp = nc.declare_dram_parameter("sh", [C, C], bf16, isOutput=False)
    outp = nc.declare_dram_parameter("outW", [C, L], bf16, isOutput=True)

    co = nc.alloc_sbuf_tensor("co", [C, SL], bf16)
    b0 = nc.alloc_sbuf_tensor("b0", [C, L], bf16)
    b1 = nc.alloc_sbuf_tensor("b1", [C, L], bf16)
    sht = nc.alloc_sbuf_tensor("sht", [C, C], bf16)
    pc0 = nc.alloc_psum_tensor("pc0", [C, 1], f32)
    pc1 = nc.alloc_psum_tensor("pc1", [C, 1], f32)

    dma_sem = nc.alloc_semaphore("dma_sem")
    dmb_sem = nc.alloc_semaphore("dmb_sem")
    step_sem = nc.alloc_semaphore("step_sem")
    out_sem = nc.alloc_semaphore("out_sem")

    bufs = [b0, b1]
    pcs = [pc0, pc1]
    TOTAL = 2 + S + (S - 2)
    HW_ = SHEAD * L                  # head columns (steps 1..SHEAD)

    with nc.Block("main", no_gpsimd_drain=True) as blk:

        @blk.sync
        def _(sync):
            sync.dma_start(out=co[:, 0:HW_], in_=cop[0][:]).then_inc(dma_sem, 16)
            sync.dma_start(out=b0[:], in_=v0p[:]).then_inc(dma_sem, 16)
            sync.wait_ge(step_sem, TOTAL)
            sync.dma_start(out=outp[:], in_=bufs[S % 2][:]).then_inc(out_sem, 16)
            sync.wait_ge(out_sem, 16)

        @blk.scalar
        def _(scalar):
            scalar.dma_start(out=sht[:], in_=shp[:]).then_inc(dma_sem, 16)
            scalar.dma_start(out=co[:, HW_:SL], in_=cop[1][:]).then_inc(dmb_sem, 16)

        @blk.tensor
        def _(tensor):
            tensor.wait_ge(dma_sem, 48)          # head coef + v0 + sh
            nc.tensor.matmul(
                pc0[:, 0:1], sht[:], sht[:, 0:1], start=True, stop=True
            ).then_inc(step_sem, 1)
            nc.tensor.matmul(
                pc1[:, 0:1], sht[:], sht[:, 0:1], start=True, stop=True
            ).then_inc(step_sem, 1)
            for s in range(1, S - 1):
                if s == SHEAD - 2:
                    # gate the coef tail; scans >= SHEAD inherit via mm chain
                    tensor.wait_ge(dmb_sem, 16)
                tensor.wait_ge(step_sem, 2 * s + 1)
                nc.tensor.matmul(
                    pcs[s % 2][:, 0:1], sht[:], bufs[s % 2][:, L - 1 : L],
                    start=True, stop=True,
                ).then_inc(step_sem, 1)

        @blk.vector
        def _(vector):
            for s in range(1, S + 1):
                vector.wait_ge(step_sem, 2 + (s - 1) + max(0, s - 2))
                nc.vector.tensor_tensor_scan(
                    out=bufs[s % 2][:],
                    data0=co[:, (s - 1) * L : s * L],
                    data1=bufs[(s - 1) % 2][:],
                    initial=pcs[s % 2][:, 0:1],
                    op0=mybir.AluOpType.mult,
                    op1=mybir.AluOpType.add,
                ).then_inc(step_sem, 1)

    _fuse_event_waits(nc, mybir)
    nc.finalize()
    _check_single_waits(nc)
    return nc


def _fuse_event_waits(nc, mybir):
    """Merge standalone wait_ge EventSemaphore instructions into the next
    same-engine instruction's sync_info (saves ~80ns/step on the DVE)."""
    f = nc.m.functions[0]
    for attr in ("basic_blocks", "bbs", "blocks"):
        if hasattr(f, attr):
            bbs = getattr(f, attr)
            break
    else:
        return
    for bb in bbs:
        insts = list(bb.instructions)
        drop = set()
        pending = {}
        for ins in insts:
            eng = getattr(ins, "engine", None)
            si = ins.sync_info
            if (
                ins.opcode == "EventSemaphore"
                and si is not None
                and len(si.on_wait) == 1
                and len(si.on_update) == 0
                and eng in (mybir.EngineType.DVE, mybir.EngineType.PE,
                            mybir.EngineType.SP)
            ):
                pending.setdefault(eng, []).append(ins)
                continue
            if eng in pending and pending[eng]:
                if ins.opcode in ("TensorScalarPtr", "Matmult", "DMACopy"):
                    waits = list(si.on_wait) if si else []
                    ups = list(si.on_update) if si else []
                    for ev in pending[eng]:
                        waits += list(ev.sync_info.on_wait)
                        drop.add(id(ev))
                    if len(waits) <= 1:
                        ins.sync_info = mybir.SyncInfo(
                            on_wait=waits, on_update=ups
                        )
                    else:
                        # cannot fuse safely; keep events standalone
                        for ev in pending[eng]:
                            drop.discard(id(ev))
                pending[eng] = []
        if drop:
            bb.instructions = [i for i in insts if id(i) not in drop]


def _check_single_waits(nc):
    f = nc.m.functions[0]
    for attr in ("basic_blocks", "bbs", "blocks"):
        if hasattr(f, attr):
            bbs = getattr(f, attr)
            break
    else:
        return
    bad = []
    for bb in bbs:
        for ins in bb.instructions:
            si = ins.sync_info
            if si is not None and len(si.on_wait) > 1 and ins.opcode in (
                "TensorScalarPtr", "Matmult"
            ):
                bad.append((ins.name, ins.opcode, len(si.on_wait)))
    assert not bad, f"multi-wait hot instructions: {bad[:5]}"


def _shift_matrix():
    import ml_dtypes
    sh = np.zeros((C, C), ml_dtypes.bfloat16)
    for c in range(1, C):
        sh[c - 1, c] = 1.0          # out[c] = carry[c-1]
    return sh


def _pack_chain(lbx, lex, nrows):
    """lbx [T, nrows+1] blank col per row (col 0 drives S0), lex [T, >=nrows]
    emit cols (row u uses col u-1).  Returns (coef [C, S*L] f32,
    v0 [C, L] f32, S0 [T] f64, Efinal [T] f64 = E_{nrows}[t])."""
    g = _g_profile()
    S0 = np.zeros(T)
    S0[1:] = np.cumsum(lbx[:-1, 0])
    S0 += g
    E = np.zeros((T, nrows + 1))
    E[:, 1:] = np.cumsum(lex[:, :nrows], axis=1)
    # d0_u[t] = exp(lbx[t-1,u] + S0[t-1]-S0[t] + E_u[t-1]-E_u[t]);  d0_u[0]=0
    d0 = np.zeros((NR + 1, T), np.float32)
    uu = np.arange(1, nrows + 1)
    ld = lbx[:-1, uu] + (S0[:-1] - S0[1:])[:, None] + E[:-1, uu] - E[1:, uu]
    d0[1 : nrows + 1, 1:] = np.exp(ld).T.astype(np.float32)
    v0 = np.exp(-g).astype(np.float32).reshape(C, L)
    coef = np.zeros((C, S * L), np.float32)
    for s in range(1, S + 1):
        for c in range(C):
            u = s - 2 * c
            if 1 <= u <= NR:
                coef[c, (s - 1) * L : s * L] = d0[u, c * L : (c + 1) * L]
    return coef, v0, S0, E[:, nrows]


def _sim_chain(coef, v0):
    """Numpy simulation of the device schedule (fp32), for validation."""
    bufs = [v0.astype(np.float32).copy(), np.zeros((C, L), np.float32)]
    carr = [np.zeros(C, np.float32), np.zeros(C, np.float32)]
    for s in range(1, S + 1):
        cur = bufs[(s - 1) % 2]
        cf = coef[:, (s - 1) * L : s * L]
        state = carr[s % 2].copy()
        out = np.empty((C, L), np.float32)
        for j in range(L):
            state = cf[:, j] * state + cur[:, j]
            out[:, j] = state
        bufs[s % 2][:] = out
        if s <= S - 2:
            carr[s % 2][1:] = out[0 : C - 1, L - 1]
            carr[s % 2][0] = 0.0
    return bufs[S % 2].reshape(-1)


_RUN_STATE = {}


def _prep(inputs):
    lp = np.asarray(inputs["log_probs"], dtype=np.float32)
    tgt = np.asarray(inputs["targets"]).astype(np.int64)
    blank = int(inputs["blank"])
    lb = lp[:, :, :, blank].astype(np.float64)                     # [B,T,U]
    le = np.take_along_axis(
        lp[:, :, : U - 1, :], tgt[:, None, :, None], axis=3
    )[..., 0].astype(np.float64)                                   # [B,T,U-1]

    in_maps, recon = [], []
    sh = _shift_matrix()
    s_ = np.arange(T - 1)
    tau = np.arange(T)

    import ml_dtypes
    bf16 = ml_dtypes.bfloat16

    HW_ = SHEAD * L

    def add_map(coef, v0):
        m = {"cop0": np.ascontiguousarray(coef[:, :HW_]).astype(bf16),
             "cop1": np.ascontiguousarray(coef[:, HW_:]).astype(bf16)}
        m["v0"] = v0.astype(bf16)
        m["sh"] = sh
        in_maps.append(m)

    for b in range(B):
        coef, v0, S0, Ef = _pack_chain(lb[b, :, : NR + 1], le[b, :, :NR], NR)
        add_map(coef, v0)
        recon.append((S0, Ef))
    for b in range(B):
        lbr = np.zeros((T, NR + 1))
        for v in range(NR + 1):
            lbr[:-1, v] = lb[b, T - 2 - s_, U - 1 - v]
        ler = np.zeros((T, NR))
        for w in range(NR - 1):
            ler[:, w] = le[b, T - 1 - tau, U - 2 - w]
        coef, v0, S0, Er = _pack_chain(lbr, ler, NR - 1)
        add_map(coef, v0)
        recon.append((S0, Er))
    return lb, le, in_maps, recon


def _combine(lb, le, recon, Wf_all):
    tau = np.arange(T)
    costs = np.empty(B, np.float32)
    for b in range(B):
        Wf = Wf_all[b].astype(np.float64)
        Wr = Wf_all[4 + b].astype(np.float64)
        S0f, Ef = recon[b]
        S0r, Er = recon[4 + b]
        with np.errstate(divide="ignore"):
            alphaf = np.log(Wf) + S0f + Ef                          # alpha[t, 48]
            base = lb[b, T - 1, U - 1]
            betar = np.log(Wr) + base + S0r + Er                    # beta-hat[tau, 47]
        beta49 = betar[T - 1 - tau]                                  # beta[t, 49]
        terms = alphaf + le[b, :, NR] + beta49
        mx = terms.max()
        costs[b] = np.float32(-(mx + np.log(np.sum(np.exp(terms - mx)))))
    return costs


def kernel(**inputs) -> np.ndarray:
    _install_shims()
    from concourse.bass_utils import run_bass_kernel_spmd

    lb, le, in_maps, recon = _prep(inputs)
    nc = _build_nc()
    r = run_bass_kernel_spmd(
        nc, in_maps, list(range(8)), trace=_RUN_STATE.get("trace", False)
    )
    _RUN_STATE["last"] = r
    Wf_all = [r.results[i]["outW"].astype(np.float32).reshape(-1) for i in range(8)]
    return _combine(lb, le, recon, Wf_all)


# revision 29
# speedup vs baseline: 1.1894x; 1.0163x over previous
"""RNN-T transducer loss on TRN2 — lag-2 skewed-wavefront blocked-scan kernel.

8 cores run 8 independent DP chains (4 sequences x {fwd rows u=1..48,
bwd rows v=1..47 reversed-coords, padded}).  Each chain's 48x512
lattice block: t axis cut into C=8 chunks of L=64; one
tensor_tensor_scan per schedule step processes cells (u, c) with
u = s - 2c on C contiguous partition lanes.  TRN2 forbids +-1
partition moves on compute engines (32-aligned bases, contiguous
windows, shared input bases), so inter-chunk carries go through the
PE: a superdiagonal [C,C] matmul shifts the carry column into PSUM,
which the scan's `initial` operand reads (PSUM is exempt from the
SBUF same-base rule).  The lag-2 skew (cell (u,c) at step u+2c) gives
the PE round-trip two steps of slack, keeping the DVE critical path
pure scans.  Inactive lanes get d0=0 coefficients: the scan
degenerates to a copy, which parks finished row-48 chunks and carries
the init row forward, so the final buffer holds the full seam row.

Transform: W_u[t] = exp(alpha[t,u] - S0[t] - E_u[t]) with E_u the
cross-row emit cumsum and S0 = alpha[t,0] + g(t), g a fitted
sqrt-envelope profile.  Cross-row scan coefficient is exactly 1, all
intermediates stay in fp32 range, and cells far below the envelope
underflow to 0 harmlessly — no mid-lattice rescaling.  Host does the
O(T*U) packing and the f64 seam combine; the device executes every
lattice cell update.
"""
import numpy as np

B, T, U, D = 4, 512, 97, 512
NR = 48                      # rows per chain (bwd pads its 48th row with zeros)
C = 8                        # t-chunks (contiguous scan lanes)
L = T // C                   # elements per chunk
S = NR + 2 * (C - 1)         # schedule steps (lag-2 skew)
NDMA = 2                     # coefficient DMA splits (issued from SP + ACT)
SHEAD = 16                   # steps covered by the head coefficient DMA
HSHIFT = 25.0                # downward shift of the envelope profile


def _g_profile():
    t = np.arange(T, dtype=np.float64)
    return 17.22 * np.sqrt(t) - 0.092 * t - 1.94 - HSHIFT


def _install_shims():
    import sys, types
    try:
        import antenv.axon_hooks  # noqa: F401
    except Exception:
        m = types.ModuleType("antenv.axon_hooks")
        m._hook = None
        m.set_axon_ntff_profile_hook = lambda h: setattr(m, "_hook", h)
        m.get_axon_ntff_profile_hook = lambda: getattr(m, "_hook", None)
        sys.modules["antenv.axon_hooks"] = m
        try:
            import antenv
            antenv.axon_hooks = m
        except Exception:
            pass
        try:
            from trn_agent_boot.trn_boot import _ntff_profile_via_ctypes
            hk = _ntff_profile_via_ctypes("/opt/axon/libaxon_pjrt.so")
            if hk is not None:
                m.set_axon_ntff_profile_hook(hk)
        except Exception:
            pass

    # Split the TileContext final-drain sem waits across multiple drain
    # instructions: the CTRL encoding holds too few wait slots and the
    # walrus backend rejects the fused drain ("Too many sync wait commands").
    import concourse.tile as _tile
    from concourse import mybir as _mybir
    from concourse.vector_clock import ScopedClock as _ScopedClock

    if getattr(_tile.TileContext, "_drain_patched", False):
        return

    def _patched_drain_and_barrier(self, tick_clock, wait_clock):
        nc = self.nc
        drain_inst = nc.sync.drain()
        wait_clock.add_sem_waits(
            drain_inst.ins, _ScopedClock({None: tick_clock.global_clock})
        )
        si = drain_inst.ins.sync_info
        waits = list(si.on_wait) if si is not None else []
        if len(waits) > 1:
            ups = list(si.on_update) if si is not None else []
            drain_inst.ins.sync_info = _mybir.SyncInfo(on_wait=waits[:1], on_update=ups)
            for i in range(1, len(waits)):
                extra = nc.sync.drain()
                extra.ins.sync_info = _mybir.SyncInfo(
                    on_wait=waits[i : i + 1], on_update=[]
                )
        nc.all_engine_barrier()
        assert self.sems is not None
        popped = nc._tile_sem_poison_stack.pop()
        assert popped is self._sem_poison
        nc.clear_and_free_semaphores(list(self.sems.allocated().values()))
        nc.all_engine_barrier()

    _tile.TileContext._drain_and_barrier = _patched_drain_and_barrier
    _tile.TileContext._drain_patched = True


def _build_nc():
    from concourse import bass, mybir

    f32 = mybir.dt.float32
    bf16 = mybir.dt.bfloat16
    nc = bass.Bass()
    SL = S * L
    W = SL // NDMA
    HW_ = SHEAD * L                  # head columns (steps 1..SHEAD)
    INIW = L + C + HW_               # packed init: [v0 | sh | coef head]
    inip = nc.declare_dram_parameter("ini", [C, INIW], bf16, isOutput=False)
    copt = nc.declare_dram_parameter("copt", [C, SL - HW_], bf16, isOutput=False)
    outp = nc.declare_dram_parameter("outW", [C, L], bf16, isOutput=True)

    big = nc.alloc_sbuf_tensor("big", [C, L + C + SL], bf16)
    b1 = nc.alloc_sbuf_tensor("b1", [C, L], bf16)
    pc0 = nc.alloc_psum_tensor("pc0", [C, 1], f32)
    pc1 = nc.alloc_psum_tensor("pc1", [C, 1], f32)

    b0 = big[:, 0:L]
    sht = big[:, L : L + C]
    co = big[:, L + C : L + C + SL]

    dma_sem = nc.alloc_semaphore("dma_sem")
    dmb_sem = nc.alloc_semaphore("dmb_sem")
    step_sem = nc.alloc_semaphore("step_sem")
    out_sem = nc.alloc_semaphore("out_sem")

    bufs = [b0, b1[:]]
    pcs = [pc0, pc1]
    TOTAL = 2 + S + (S - 2)

    with nc.Block("main", no_gpsimd_drain=True) as blk:

        @blk.sync
        def _(sync):
            sync.dma_start(out=big[:, 0:INIW], in_=inip[:]).then_inc(dma_sem, 16)
            sync.wait_ge(step_sem, TOTAL)
            sync.dma_start(out=outp[:], in_=bufs[S % 2]).then_inc(out_sem, 16)
            sync.wait_ge(out_sem, 16)

        @blk.scalar
        def _(scalar):
            scalar.dma_start(out=big[:, INIW:], in_=copt[:]).then_inc(dmb_sem, 16)

        @blk.tensor
        def _(tensor):
            tensor.wait_ge(dma_sem, 16)          # packed init DMA
            nc.tensor.matmul(
                pc0[:, 0:1], sht, sht[:, 0:1], start=True, stop=True
            ).then_inc(step_sem, 1)
            nc.tensor.matmul(
                pc1[:, 0:1], sht, sht[:, 0:1], start=True, stop=True
            ).then_inc(step_sem, 1)
            for s in range(1, S - 1):
                if s == SHEAD - 2:
                    # gate the coef tail; scans >= SHEAD inherit via mm chain
                    tensor.wait_ge(dmb_sem, 16)
                tensor.wait_ge(step_sem, 2 * s + 1)
                nc.tensor.matmul(
                    pcs[s % 2][:, 0:1], sht, bufs[s % 2][:, L - 1 : L],
                    start=True, stop=True,
                ).then_inc(step_sem, 1)

        @blk.vector
        def _(vector):
            for s in range(1, S + 1):
                vector.wait_ge(step_sem, 2 + (s - 1) + max(0, s - 2))
                nc.vector.tensor_tensor_scan(
                    out=bufs[s % 2],
                    data0=co[:, (s - 1) * L : s * L],
                    data1=bufs[(s - 1) % 2],
                    initial=pcs[s % 2][:, 0:1],
                    op0=mybir.AluOpType.mult,
                    op1=mybir.AluOpType.add,
                ).then_inc(step_sem, 1)

    _fuse_event_waits(nc, mybir)
    nc.finalize()
    _check_single_waits(nc)
    return nc


def _fuse_event_waits(nc, mybir):
    """Merge standalone wait_ge EventSemaphore instructions into the next
    same-engine instruction's sync_info (saves ~80ns/step on the DVE)."""
    f = nc.m.functions[0]
    for attr in ("basic_blocks", "bbs", "blocks"):
        if hasattr(f, attr):
            bbs = getattr(f, attr)
            break
    else:
        return
    for bb in bbs:
        insts = list(bb.instructions)
        drop = set()
        pending = {}
        for ins in insts:
            eng = getattr(ins, "engine", None)
            si = ins.sync_info
            if (
                ins.opcode == "EventSemaphore"
                and si is not None
                and len(si.on_wait) == 1
                and len(si.on_update) == 0
                and eng in (mybir.EngineType.DVE, mybir.EngineType.PE,
                            mybir.EngineType.SP)
            ):
                pending.setdefault(eng, []).append(ins)
                continue
            if eng in pending and pending[eng]:
                if ins.opcode in ("TensorScalarPtr", "Matmult", "DMACopy"):
                    waits = list(si.on_wait) if si else []
                    ups = list(si.on_update) if si else []
                    for ev in pending[eng]:
                        waits += list(ev.sync_info.on_wait)
                        drop.add(id(ev))
                    if len(waits) <= 1:
                        ins.sync_info = mybir.SyncInfo(
                            on_wait=waits, on_update=ups
                        )
                    else:
                        # cannot fuse safely; keep events standalone
                        for ev in pending[eng]:
                            drop.discard(id(ev))
                pending[eng] = []
        if drop:
            bb.instructions = [i for i in insts if id(i) not in drop]


def _check_single_waits(nc):
    f = nc.m.functions[0]
    for attr in ("basic_blocks", "bbs", "blocks"):
        if hasattr(f, attr):
            bbs = getattr(f, attr)
            break
    else:
        return
    bad = []
    for bb in bbs:
        for ins in bb.instructions:
            si = ins.sync_info
            if si is not None and len(si.on_wait) > 1 and ins.opcode in (
                "TensorScalarPtr", "Matmult"
            ):
                bad.append((ins.name, ins.opcode, len(si.on_wait)))
    assert not bad, f"multi-wait hot instructions: {bad[:5]}"


def _shift_matrix():
    import ml_dtypes
    sh = np.zeros((C, C), ml_dtypes.bfloat16)
    for c in range(1, C):
        sh[c - 1, c] = 1.0          # out[c] = carry[c-1]
    return sh


def _pack_chain(lbx, lex, nrows):
    """lbx [T, nrows+1] blank col per row (col 0 drives S0), lex [T, >=nrows]
    emit cols (row u uses col u-1).  Returns (coef [C, S*L] f32,
    v0 [C, L] f32, S0 [T] f64, Efinal [T] f64 = E_{nrows}[t])."""
    g = _g_profile()
    S0 = np.zeros(T)
    S0[1:] = np.cumsum(lbx[:-1, 0])
    S0 += g
    E = np.zeros((T, nrows + 1))
    E[:, 1:] = np.cumsum(lex[:, :nrows], axis=1)
    # d0_u[t] = exp(lbx[t-1,u] + S0[t-1]-S0[t] + E_u[t-1]-E_u[t]);  d0_u[0]=0
    d0 = np.zeros((NR + 1, T), np.float32)
    uu = np.arange(1, nrows + 1)
    ld = lbx[:-1, uu] + (S0[:-1] - S0[1:])[:, None] + E[:-1, uu] - E[1:, uu]
    d0[1 : nrows + 1, 1:] = np.exp(ld).T.astype(np.float32)
    v0 = np.exp(-g).astype(np.float32).reshape(C, L)
    coef = np.zeros((C, S * L), np.float32)
    for s in range(1, S + 1):
        for c in range(C):
            u = s - 2 * c
            if 1 <= u <= NR:
                coef[c, (s - 1) * L : s * L] = d0[u, c * L : (c + 1) * L]
    return coef, v0, S0, E[:, nrows]


def _sim_chain(coef, v0):
    """Numpy simulation of the device schedule (fp32), for validation."""
    bufs = [v0.astype(np.float32).copy(), np.zeros((C, L), np.float32)]
    carr = [np.zeros(C, np.float32), np.zeros(C, np.float32)]
    for s in range(1, S + 1):
        cur = bufs[(s - 1) % 2]
        cf = coef[:, (s - 1) * L : s * L]
        state = carr[s % 2].copy()
        out = np.empty((C, L), np.float32)
        for j in range(L):
            state = cf[:, j] * state + cur[:, j]
            out[:, j] = state
        bufs[s % 2][:] = out
        if s <= S - 2:
            carr[s % 2][1:] = out[0 : C - 1, L - 1]
            carr[s % 2][0] = 0.0
    return bufs[S % 2].reshape(-1)


_RUN_STATE = {}


def _prep(inputs):
    lp = np.asarray(inputs["log_probs"], dtype=np.float32)
    tgt = np.asarray(inputs["targets"]).astype(np.int64)
    blank = int(inputs["blank"])
    lb = lp[:, :, :, blank].astype(np.float64)                     # [B,T,U]
    le = np.take_along_axis(
        lp[:, :, : U - 1, :], tgt[:, None, :, None], axis=3
    )[..., 0].astype(np.float64)                                   # [B,T,U-1]

    in_maps, recon = [], []
    sh = _shift_matrix()
    s_ = np.arange(T - 1)
    tau = np.arange(T)

    import ml_dtypes
    bf16 = ml_dtypes.bfloat16

    HW_ = SHEAD * L

    def add_map(coef, v0):
        ini = np.concatenate(
            [v0.astype(bf16), sh, coef[:, :HW_].astype(bf16)], axis=1
        )
        m = {"ini": np.ascontiguousarray(ini),
             "copt": np.ascontiguousarray(coef[:, HW_:]).astype(bf16)}
        in_maps.append(m)

    for b in range(B):
        coef, v0, S0, Ef = _pack_chain(lb[b, :, : NR + 1], le[b, :, :NR], NR)
        add_map(coef, v0)
        recon.append((S0, Ef))
    for b in range(B):
        lbr = np.zeros((T, NR + 1))
        for v in range(NR + 1):
            lbr[:-1, v] = lb[b, T - 2 - s_, U - 1 - v]
        ler = np.zeros((T, NR))
        for w in range(NR - 1):
            ler[:, w] = le[b, T - 1 - tau, U - 2 - w]
        coef, v0, S0, Er = _pack_chain(lbr, ler, NR - 1)
        add_map(coef, v0)
        recon.append((S0, Er))
    return lb, le, in_maps, recon


def _combine(lb, le, recon, Wf_all):
    tau = np.arange(T)
    costs = np.empty(B, np.float32)
    for b in range(B):
        Wf = Wf_all[b].astype(np.float64)
        Wr = Wf_all[4 + b].astype(np.float64)
        S0f, Ef = recon[b]
        S0r, Er = recon[4 + b]
        with np.errstate(divide="ignore"):
            alphaf = np.log(Wf) + S0f + Ef                          # alpha[t, 48]
            base = lb[b, T - 1, U - 1]
            betar = np.log(Wr) + base + S0r + Er                    # beta-hat[tau, 47]
        beta49 = betar[T - 1 - tau]                                  # beta[t, 49]
        terms = alphaf + le[b, :, NR] + beta49
        mx = terms.max()
        costs[b] = np.float32(-(mx + np.log(np.sum(np.exp(terms - mx)))))
    return costs


def kernel(**inputs) -> np.ndarray:
    _install_shims()
    from concourse.bass_utils import run_bass_kernel_spmd

    lb, le, in_maps, recon = _prep(inputs)
    nc = _build_nc()
    r = run_bass_kernel_spmd(
        nc, in_maps, list(range(8)), trace=_RUN_STATE.get("trace", False)
    )
    _RUN_STATE["last"] = r
    Wf_all = [r.results[i]["outW"].astype(np.float32).reshape(-1) for i in range(8)]
    return _combine(lb, le, recon, Wf_all)


# revision 36
# speedup vs baseline: 1.1911x; 1.0014x over previous
"""RNN-T transducer loss on TRN2 — lag-2 skewed-wavefront blocked-scan kernel.

8 cores run 8 independent DP chains (4 sequences x {fwd rows u=1..48,
bwd rows v=1..47 reversed-coords, padded}).  Each chain's 48x512
lattice block: t axis cut into C=8 chunks of L=64; one
tensor_tensor_scan per schedule step processes cells (u, c) with
u = s - 2c on C contiguous partition lanes.  TRN2 forbids +-1
partition moves on compute engines (32-aligned bases, contiguous
windows, shared input bases), so inter-chunk carries go through the
PE: a superdiagonal [C,C] matmul shifts the carry column into PSUM,
which the scan's `initial` operand reads (PSUM is exempt from the
SBUF same-base rule).  The lag-2 skew (cell (u,c) at step u+2c) gives
the PE round-trip two steps of slack, keeping the DVE critical path
pure scans.  Inactive lanes get d0=0 coefficients: the scan
degenerates to a copy, which parks finished row-48 chunks and carries
the init row forward, so the final buffer holds the full seam row.

Transform: W_u[t] = exp(alpha[t,u] - S0[t] - E_u[t]) with E_u the
cross-row emit cumsum and S0 = alpha[t,0] + g(t), g a fitted
sqrt-envelope profile.  Cross-row scan coefficient is exactly 1, all
intermediates stay in fp32 range, and cells far below the envelope
underflow to 0 harmlessly — no mid-lattice rescaling.  Host does the
O(T*U) packing and the f64 seam combine; the device executes every
lattice cell update.
"""
import numpy as np

B, T, U, D = 4, 512, 97, 512
NR = 48                      # rows per chain (bwd pads its 48th row with zeros)
C = 8                        # t-chunks (contiguous scan lanes)
L = T // C                   # elements per chunk
S = NR + 2 * (C - 1)         # schedule steps (lag-2 skew)
NDMA = 2                     # coefficient DMA splits (issued from SP + ACT)
SHEAD = 16                   # steps covered by the head coefficient DMA
HSHIFT = 25.0                # downward shift of the envelope profile


def _g_profile():
    t = np.arange(T, dtype=np.float64)
    return 17.22 * np.sqrt(t) - 0.092 * t - 1.94 - HSHIFT


def _install_shims():
    import sys, types
    try:
        import antenv.axon_hooks  # noqa: F401
    except Exception:
        m = types.ModuleType("antenv.axon_hooks")
        m._hook = None
        m.set_axon_ntff_profile_hook = lambda h: setattr(m, "_hook", h)
        m.get_axon_ntff_profile_hook = lambda: getattr(m, "_hook", None)
        sys.modules["antenv.axon_hooks"] = m
        try:
            import antenv
            antenv.axon_hooks = m
        except Exception:
            pass
        try:
            from trn_agent_boot.trn_boot import _ntff_profile_via_ctypes
            hk = _ntff_profile_via_ctypes("/opt/axon/libaxon_pjrt.so")
            if hk is not None:
                m.set_axon_ntff_profile_hook(hk)
        except Exception:
            pass

    # Split the TileContext final-drain sem waits across multiple drain
    # instructions: the CTRL encoding holds too few wait slots and the
    # walrus backend rejects the fused drain ("Too many sync wait commands").
    import concourse.tile as _tile
    from concourse import mybir as _mybir
    from concourse.vector_clock import ScopedClock as _ScopedClock

    if getattr(_tile.TileContext, "_drain_patched", False):
        return

    def _patched_drain_and_barrier(self, tick_clock, wait_clock):
        nc = self.nc
        drain_inst = nc.sync.drain()
        wait_clock.add_sem_waits(
            drain_inst.ins, _ScopedClock({None: tick_clock.global_clock})
        )
        si = drain_inst.ins.sync_info
        waits = list(si.on_wait) if si is not None else []
        if len(waits) > 1:
            ups = list(si.on_update) if si is not None else []
            drain_inst.ins.sync_info = _mybir.SyncInfo(on_wait=waits[:1], on_update=ups)
            for i in range(1, len(waits)):
                extra = nc.sync.drain()
                extra.ins.sync_info = _mybir.SyncInfo(
                    on_wait=waits[i : i + 1], on_update=[]
                )
        nc.all_engine_barrier()
        assert self.sems is not None
        popped = nc._tile_sem_poison_stack.pop()
        assert popped is self._sem_poison
        nc.clear_and_free_semaphores(list(self.sems.allocated().values()))
        nc.all_engine_barrier()

    _tile.TileContext._drain_and_barrier = _patched_drain_and_barrier
    _tile.TileContext._drain_patched = True


def _build_nc():
    from concourse import bass, mybir

    f32 = mybir.dt.float32
    bf16 = mybir.dt.bfloat16
    nc = bass.Bass()
    SL = S * L
    W = SL // NDMA
    HW_ = SHEAD * L                  # head columns (steps 1..SHEAD)
    INIW = L + C + HW_               # packed init: [v0 | sh | coef head]
    inip = nc.declare_dram_parameter("ini", [C, INIW], bf16, isOutput=False)
    copt = nc.declare_dram_parameter("copt", [C, SL - HW_], bf16, isOutput=False)
    outp = nc.declare_dram_parameter("outW", [C, L], bf16, isOutput=True)

    big = nc.alloc_sbuf_tensor("big", [C, L + C + SL], bf16)
    b1 = nc.alloc_sbuf_tensor("b1", [C, L], bf16)
    pc0 = nc.alloc_psum_tensor("pc0", [C, 1], f32)
    pc1 = nc.alloc_psum_tensor("pc1", [C, 1], f32)

    b0 = big[:, 0:L]
    sht = big[:, L : L + C]
    co = big[:, L + C : L + C + SL]

    dma_sem = nc.alloc_semaphore("dma_sem")
    dmb_sem = nc.alloc_semaphore("dmb_sem")
    step_sem = nc.alloc_semaphore("step_sem")
    out_sem = nc.alloc_semaphore("out_sem")

    bufs = [b0, b1[:]]
    pcs = [pc0, pc1]
    TOTAL = 2 + S + (S - 2)

    with nc.Block("main", no_gpsimd_drain=True) as blk:

        @blk.sync
        def _(sync):
            sync.dma_start(out=big[:, 0:INIW], in_=inip[:]).then_inc(dma_sem, 16)
            sync.wait_ge(step_sem, TOTAL)
            sync.dma_start(out=outp[:], in_=bufs[S % 2]).then_inc(out_sem, 16)
            sync.wait_ge(out_sem, 16)

        @blk.scalar
        def _(scalar):
            scalar.dma_start(out=big[:, INIW:], in_=copt[:]).then_inc(dmb_sem, 16)

        @blk.tensor
        def _(tensor):
            tensor.wait_ge(dma_sem, 16)          # packed init DMA
            nc.tensor.matmul(
                pc0[:, 0:1], sht, sht[:, 0:1], start=True, stop=True
            ).then_inc(step_sem, 1)
            nc.tensor.matmul(
                pc1[:, 0:1], sht, sht[:, 0:1], start=True, stop=True
            ).then_inc(step_sem, 1)
            for s in range(1, S - 1):
                if s == SHEAD - 2:
                    # gate the coef tail; scans >= SHEAD inherit via mm chain
                    tensor.wait_ge(dmb_sem, 16)
                tensor.wait_ge(step_sem, 2 * s + 1)
                nc.tensor.matmul(
                    pcs[s % 2][:, 0:1], sht, bufs[s % 2][:, L - 1 : L],
                    start=True, stop=True,
                ).then_inc(step_sem, 1)

        @blk.vector
        def _(vector):
            for s in range(1, S + 1):
                vector.wait_ge(step_sem, 2 + (s - 1) + max(0, s - 2))
                nc.vector.tensor_tensor_scan(
                    out=bufs[s % 2],
                    data0=co[:, (s - 1) * L : s * L],
                    data1=bufs[(s - 1) % 2],
                    initial=pcs[s % 2][:, 0:1],
                    op0=mybir.AluOpType.mult,
                    op1=mybir.AluOpType.add,
                ).then_inc(step_sem, 1)

    _fuse_event_waits(nc, mybir)
    nc.finalize()
    _check_single_waits(nc)
    return nc


def _fuse_event_waits(nc, mybir):
    """Merge standalone wait_ge EventSemaphore instructions into the next
    same-engine instruction's sync_info (saves ~80ns/step on the DVE)."""
    f = nc.m.functions[0]
    for attr in ("basic_blocks", "bbs", "blocks"):
        if hasattr(f, attr):
            bbs = getattr(f, attr)
            break
    else:
        return
    for bb in bbs:
        insts = list(bb.instructions)
        drop = set()
        pending = {}
        for ins in insts:
            eng = getattr(ins, "engine", None)
            si = ins.sync_info
            if (
                ins.opcode == "EventSemaphore"
                and si is not None
                and len(si.on_wait) == 1
                and len(si.on_update) == 0
                and eng in (mybir.EngineType.DVE, mybir.EngineType.PE,
                            mybir.EngineType.SP)
            ):
                pending.setdefault(eng, []).append(ins)
                continue
            if eng in pending and pending[eng]:
                if ins.opcode in ("TensorScalarPtr", "Matmult", "DMACopy"):
                    waits = list(si.on_wait) if si else []
                    ups = list(si.on_update) if si else []
                    for ev in pending[eng]:
                        waits += list(ev.sync_info.on_wait)
                        drop.add(id(ev))
                    if len(waits) <= 1:
                        ins.sync_info = mybir.SyncInfo(
                            on_wait=waits, on_update=ups
                        )
                    else:
                        # cannot fuse safely; keep events standalone
                        for ev in pending[eng]:
                            drop.discard(id(ev))
                pending[eng] = []
        if drop:
            bb.instructions = [i for i in insts if id(i) not in drop]


def _check_single_waits(nc):
    f = nc.m.functions[0]
    for attr in ("basic_blocks", "bbs", "blocks"):
        if hasattr(f, attr):
            bbs = getattr(f, attr)
            break
    else:
        return
    bad = []
    for bb in bbs:
        for ins in bb.instructions:
            si = ins.sync_info
            if si is not None and len(si.on_wait) > 1 and ins.opcode in (
                "TensorScalarPtr", "Matmult"
            ):
                bad.append((ins.name, ins.opcode, len(si.on_wait)))
    assert not bad, f"multi-wait hot instructions: {bad[:5]}"


def _shift_matrix():
    import ml_dtypes
    sh = np.zeros((C, C), ml_dtypes.bfloat16)
    for c in range(1, C):
        sh[c - 1, c] = 1.0          # out[c] = carry[c-1]
    return sh


def _pack_chain(lbx, lex, nrows):
    """lbx [T, nrows+1] blank col per row (col 0 drives S0), lex [T, >=nrows]
    emit cols (row u uses col u-1).  Returns (coef [C, S*L] f32,
    v0 [C, L] f32, S0 [T] f64, Efinal [T] f64 = E_{nrows}[t])."""
    g = _g_profile()
    S0 = np.zeros(T)
    S0[1:] = np.cumsum(lbx[:-1, 0])
    S0 += g
    E = np.zeros((T, nrows + 1))
    E[:, 1:] = np.cumsum(lex[:, :nrows], axis=1)
    # d0_u[t] = exp(lbx[t-1,u] + S0[t-1]-S0[t] + E_u[t-1]-E_u[t]);  d0_u[0]=0
    d0 = np.zeros((NR + 1, T), np.float32)
    uu = np.arange(1, nrows + 1)
    ld = lbx[:-1, uu] + (S0[:-1] - S0[1:])[:, None] + E[:-1, uu] - E[1:, uu]
    d0[1 : nrows + 1, 1:] = np.exp(ld).T.astype(np.float32)
    v0 = np.exp(-g).astype(np.float32).reshape(C, L)
    coef = np.zeros((C, S * L), np.float32)
    for s in range(1, S + 1):
        for c in range(C):
            u = s - 2 * c
            if 1 <= u <= NR:
                coef[c, (s - 1) * L : s * L] = d0[u, c * L : (c + 1) * L]
    return coef, v0, S0, E[:, nrows]


def _sim_chain(coef, v0):
    """Numpy simulation of the device schedule (fp32), for validation."""
    bufs = [v0.astype(np.float32).copy(), np.zeros((C, L), np.float32)]
    carr = [np.zeros(C, np.float32), np.zeros(C, np.float32)]
    for s in range(1, S + 1):
        cur = bufs[(s - 1) % 2]
        cf = coef[:, (s - 1) * L : s * L]
        state = carr[s % 2].copy()
        out = np.empty((C, L), np.float32)
        for j in range(L):
            state = cf[:, j] * state + cur[:, j]
            out[:, j] = state
        bufs[s % 2][:] = out
        if s <= S - 2:
            carr[s % 2][1:] = out[0 : C - 1, L - 1]
            carr[s % 2][0] = 0.0
    return bufs[S % 2].reshape(-1)


_RUN_STATE = {}


def _prep(inputs):
    lp = np.asarray(inputs["log_probs"], dtype=np.float32)
    tgt = np.asarray(inputs["targets"]).astype(np.int64)
    blank = int(inputs["blank"])
    lb = lp[:, :, :, blank].astype(np.float64)                     # [B,T,U]
    le = np.take_along_axis(
        lp[:, :, : U - 1, :], tgt[:, None, :, None], axis=3
    )[..., 0].astype(np.float64)                                   # [B,T,U-1]

    in_maps, recon = [], []
    sh = _shift_matrix()
    s_ = np.arange(T - 1)
    tau = np.arange(T)

    import ml_dtypes
    bf16 = ml_dtypes.bfloat16

    HW_ = SHEAD * L

    def add_map(coef, v0):
        ini = np.concatenate(
            [v0.astype(bf16), sh, coef[:, :HW_].astype(bf16)], axis=1
        )
        m = {"ini": np.ascontiguousarray(ini),
             "copt": np.ascontiguousarray(coef[:, HW_:]).astype(bf16)}
        in_maps.append(m)

    for b in range(B):
        coef, v0, S0, Ef = _pack_chain(lb[b, :, : NR + 1], le[b, :, :NR], NR)
        add_map(coef, v0)
        recon.append((S0, Ef))
    for b in range(B):
        lbr = np.zeros((T, NR + 1))
        for v in range(NR + 1):
            lbr[:-1, v] = lb[b, T - 2 - s_, U - 1 - v]
        ler = np.zeros((T, NR))
        for w in range(NR - 1):
            ler[:, w] = le[b, T - 1 - tau, U - 2 - w]
        coef, v0, S0, Er = _pack_chain(lbr, ler, NR - 1)
        add_map(coef, v0)
        recon.append((S0, Er))
    return lb, le, in_maps, recon


def _combine(lb, le, recon, Wf_all):
    tau = np.arange(T)
    costs = np.empty(B, np.float32)
    for b in range(B):
        Wf = Wf_all[b].astype(np.float64)
        Wr = Wf_all[4 + b].astype(np.float64)
        S0f, Ef = recon[b]
        S0r, Er = recon[4 + b]
        with np.errstate(divide="ignore"):
            alphaf = np.log(Wf) + S0f + Ef                          # alpha[t, 48]
            base = lb[b, T - 1, U - 1]
            betar = np.log(Wr) + base + S0r + Er                    # beta-hat[tau, 47]
        beta49 = betar[T - 1 - tau]                                  # beta[t, 49]
        terms = alphaf + le[b, :, NR] + beta49
        mx = terms.max()
        costs[b] = np.float32(-(mx + np.log(np.sum(np.exp(terms - mx)))))
    return costs


def kernel(**inputs) -> np.ndarray:
    _install_shims()
    from concourse.bass_utils import run_bass_kernel_spmd

    lb, le, in_maps, recon = _prep(inputs)
    nc = _build_nc()
    r = run_bass_kernel_spmd(
        nc, in_maps, list(range(8)), trace=_RUN_STATE.get("trace", False)
    )
    _RUN_STATE["last"] = r
    Wf_all = [r.results[i]["outW"].astype(np.float32).reshape(-1) for i in range(8)]
    return _combine(lb, le, recon, Wf_all)


# revision 38
# speedup vs baseline: 1.1979x; 1.0057x over previous
"""RNN-T transducer loss on TRN2 — lag-2 skewed-wavefront blocked-scan kernel.

8 cores run 8 independent DP chains (4 sequences x {fwd rows u=1..48,
bwd rows v=1..47 reversed-coords, padded}).  Each chain's 48x512
lattice block: t axis cut into C=8 chunks of L=64; one
tensor_tensor_scan per schedule step processes cells (u, c) with
u = s - 2c on C contiguous partition lanes.  TRN2 forbids +-1
partition moves on compute engines (32-aligned bases, contiguous
windows, shared input bases), so inter-chunk carries go through the
PE: a superdiagonal [C,C] matmul shifts the carry column into PSUM,
which the scan's `initial` operand reads (PSUM is exempt from the
SBUF same-base rule).  The lag-2 skew (cell (u,c) at step u+2c) gives
the PE round-trip two steps of slack, keeping the DVE critical path
pure scans.  Inactive lanes get d0=0 coefficients: the scan
degenerates to a copy, which parks finished row-48 chunks and carries
the init row forward, so the final buffer holds the full seam row.

Transform: W_u[t] = exp(alpha[t,u] - S0[t] - E_u[t]) with E_u the
cross-row emit cumsum and S0 = alpha[t,0] + g(t), g a fitted
sqrt-envelope profile.  Cross-row scan coefficient is exactly 1, all
intermediates stay in fp32 range, and cells far below the envelope
underflow to 0 harmlessly — no mid-lattice rescaling.  Host does the
O(T*U) packing and the f64 seam combine; the device executes every
lattice cell update.
"""
import numpy as np

B, T, U, D = 4, 512, 97, 512
NR = 48                      # rows per chain (bwd pads its 48th row with zeros)
C = 8                        # t-chunks (contiguous scan lanes)
L = T // C                   # elements per chunk
S = NR + 2 * (C - 1)         # schedule steps (lag-2 skew)
NDMA = 2                     # coefficient DMA splits (issued from SP + ACT)
SHEAD = 8                    # steps covered by the head coefficient DMA
HSHIFT = 25.0                # downward shift of the envelope profile


def _g_profile():
    t = np.arange(T, dtype=np.float64)
    return 17.22 * np.sqrt(t) - 0.092 * t - 1.94 - HSHIFT


def _install_shims():
    import sys, types
    try:
        import antenv.axon_hooks  # noqa: F401
    except Exception:
        m = types.ModuleType("antenv.axon_hooks")
        m._hook = None
        m.set_axon_ntff_profile_hook = lambda h: setattr(m, "_hook", h)
        m.get_axon_ntff_profile_hook = lambda: getattr(m, "_hook", None)
        sys.modules["antenv.axon_hooks"] = m
        try:
            import antenv
            antenv.axon_hooks = m
        except Exception:
            pass
        try:
            from trn_agent_boot.trn_boot import _ntff_profile_via_ctypes
            hk = _ntff_profile_via_ctypes("/opt/axon/libaxon_pjrt.so")
            if hk is not None:
                m.set_axon_ntff_profile_hook(hk)
        except Exception:
            pass

    # Split the TileContext final-drain sem waits across multiple drain
    # instructions: the CTRL encoding holds too few wait slots and the
    # walrus backend rejects the fused drain ("Too many sync wait commands").
    import concourse.tile as _tile
    from concourse import mybir as _mybir
    from concourse.vector_clock import ScopedClock as _ScopedClock

    if getattr(_tile.TileContext, "_drain_patched", False):
        return

    def _patched_drain_and_barrier(self, tick_clock, wait_clock):
        nc = self.nc
        drain_inst = nc.sync.drain()
        wait_clock.add_sem_waits(
            drain_inst.ins, _ScopedClock({None: tick_clock.global_clock})
        )
        si = drain_inst.ins.sync_info
        waits = list(si.on_wait) if si is not None else []
        if len(waits) > 1:
            ups = list(si.on_update) if si is not None else []
            drain_inst.ins.sync_info = _mybir.SyncInfo(on_wait=waits[:1], on_update=ups)
            for i in range(1, len(waits)):
                extra = nc.sync.drain()
                extra.ins.sync_info = _mybir.SyncInfo(
                    on_wait=waits[i : i + 1], on_update=[]
                )
        nc.all_engine_barrier()
        assert self.sems is not None
        popped = nc._tile_sem_poison_stack.pop()
        assert popped is self._sem_poison
        nc.clear_and_free_semaphores(list(self.sems.allocated().values()))
        nc.all_engine_barrier()

    _tile.TileContext._drain_and_barrier = _patched_drain_and_barrier
    _tile.TileContext._drain_patched = True


def _build_nc():
    from concourse import bass, mybir

    f32 = mybir.dt.float32
    bf16 = mybir.dt.bfloat16
    nc = bass.Bass()
    SL = S * L
    W = SL // NDMA
    HW_ = SHEAD * L                  # head columns (steps 1..SHEAD)
    INIW = L + HW_                   # packed init: [v0 | coef head]
    inip = nc.declare_dram_parameter("ini", [C, INIW], bf16, isOutput=False)
    shp = nc.declare_dram_parameter("sh", [C, C], bf16, isOutput=False)
    copt = nc.declare_dram_parameter("copt", [C, SL - HW_], bf16, isOutput=False)
    outp = nc.declare_dram_parameter("outW", [C, L], bf16, isOutput=True)

    big = nc.alloc_sbuf_tensor("big", [C, L + SL], bf16)
    b1 = nc.alloc_sbuf_tensor("b1", [C, L], bf16)
    shT = nc.alloc_sbuf_tensor("shT", [C, C], bf16)
    pc0 = nc.alloc_psum_tensor("pc0", [C, 1], f32)
    pc1 = nc.alloc_psum_tensor("pc1", [C, 1], f32)

    b0 = big[:, 0:L]
    sht = shT[:]
    co = big[:, L : L + SL]

    shs_sem = nc.alloc_semaphore("shs_sem")
    dma_sem = nc.alloc_semaphore("dma_sem")
    dmb_sem = nc.alloc_semaphore("dmb_sem")
    step_sem = nc.alloc_semaphore("step_sem")
    out_sem = nc.alloc_semaphore("out_sem")

    bufs = [b0, b1[:]]
    pcs = [pc0, pc1]
    TOTAL = 2 + S + (S - 2)

    with nc.Block("main", no_gpsimd_drain=True) as blk:

        @blk.sync
        def _(sync):
            sync.dma_start(out=big[:, 0:INIW], in_=inip[:]).then_inc(dma_sem, 16)
            sync.wait_ge(step_sem, TOTAL)
            sync.dma_start(out=outp[:], in_=bufs[S % 2]).then_inc(out_sem, 16)
            sync.wait_ge(out_sem, 16)

        @blk.scalar
        def _(scalar):
            scalar.dma_start(out=sht, in_=shp[:]).then_inc(shs_sem, 16)
            scalar.dma_start(out=big[:, INIW:], in_=copt[:]).then_inc(dmb_sem, 16)

        @blk.tensor
        def _(tensor):
            tensor.wait_ge(shs_sem, 16)          # shift matrix only
            nc.tensor.matmul(
                pc0[:, 0:1], sht, sht[:, 0:1], start=True, stop=True
            ).then_inc(step_sem, 1)
            nc.tensor.matmul(
                pc1[:, 0:1], sht, sht[:, 0:1], start=True, stop=True
            ).then_inc(step_sem, 1)
            for s in range(1, S - 1):
                if s == SHEAD - 2:
                    # gate the coef tail; scans >= SHEAD inherit via mm chain
                    tensor.wait_ge(dmb_sem, 16)
                tensor.wait_ge(step_sem, 2 * s + 1)
                nc.tensor.matmul(
                    pcs[s % 2][:, 0:1], sht, bufs[s % 2][:, L - 1 : L],
                    start=True, stop=True,
                ).then_inc(step_sem, 1)

        @blk.vector
        def _(vector):
            vector.wait_ge(dma_sem, 16)          # packed [v0 | coef head] DMA
            for s in range(1, S + 1):
                vector.wait_ge(step_sem, 2 + (s - 1) + max(0, s - 2))
                nc.vector.tensor_tensor_scan(
                    out=bufs[s % 2],
                    data0=co[:, (s - 1) * L : s * L],
                    data1=bufs[(s - 1) % 2],
                    initial=pcs[s % 2][:, 0:1],
                    op0=mybir.AluOpType.mult,
                    op1=mybir.AluOpType.add,
                ).then_inc(step_sem, 1)

    _fuse_event_waits(nc, mybir)
    nc.finalize()
    _check_single_waits(nc)
    return nc


def _fuse_event_waits(nc, mybir):
    """Merge standalone wait_ge EventSemaphore instructions into the next
    same-engine instruction's sync_info (saves ~80ns/step on the DVE)."""
    f = nc.m.functions[0]
    for attr in ("basic_blocks", "bbs", "blocks"):
        if hasattr(f, attr):
            bbs = getattr(f, attr)
            break
    else:
        return
    for bb in bbs:
        insts = list(bb.instructions)
        drop = set()
        pending = {}
        for ins in insts:
            eng = getattr(ins, "engine", None)
            si = ins.sync_info
            if (
                ins.opcode == "EventSemaphore"
                and si is not None
                and len(si.on_wait) == 1
                and len(si.on_update) == 0
                and eng in (mybir.EngineType.DVE, mybir.EngineType.PE,
                            mybir.EngineType.SP)
            ):
                pending.setdefault(eng, []).append(ins)
                continue
            if eng in pending and pending[eng]:
                if ins.opcode in ("TensorScalarPtr", "Matmult", "DMACopy"):
                    waits = list(si.on_wait) if si else []
                    ups = list(si.on_update) if si else []
                    for ev in pending[eng]:
                        waits += list(ev.sync_info.on_wait)
                        drop.add(id(ev))
                    if len(waits) <= 1:
                        ins.sync_info = mybir.SyncInfo(
                            on_wait=waits, on_update=ups
                        )
                    else:
                        # cannot fuse safely; keep events standalone
                        for ev in pending[eng]:
                            drop.discard(id(ev))
                pending[eng] = []
        if drop:
            bb.instructions = [i for i in insts if id(i) not in drop]


def _check_single_waits(nc):
    f = nc.m.functions[0]
    for attr in ("basic_blocks", "bbs", "blocks"):
        if hasattr(f, attr):
            bbs = getattr(f, attr)
            break
    else:
        return
    bad = []
    for bb in bbs:
        for ins in bb.instructions:
            si = ins.sync_info
            if si is not None and len(si.on_wait) > 1 and ins.opcode in (
                "TensorScalarPtr", "Matmult"
            ):
                bad.append((ins.name, ins.opcode, len(si.on_wait)))
    assert not bad, f"multi-wait hot instructions: {bad[:5]}"


def _shift_matrix():
    import ml_dtypes
    sh = np.zeros((C, C), ml_dtypes.bfloat16)
    for c in range(1, C):
        sh[c - 1, c] = 1.0          # out[c] = carry[c-1]
    return sh


def _pack_chain(lbx, lex, nrows):
    """lbx [T, nrows+1] blank col per row (col 0 drives S0), lex [T, >=nrows]
    emit cols (row u uses col u-1).  Returns (coef [C, S*L] f32,
    v0 [C, L] f32, S0 [T] f64, Efinal [T] f64 = E_{nrows}[t])."""
    g = _g_profile()
    S0 = np.zeros(T)
    S0[1:] = np.cumsum(lbx[:-1, 0])
    S0 += g
    E = np.zeros((T, nrows + 1))
    E[:, 1:] = np.cumsum(lex[:, :nrows], axis=1)
    # d0_u[t] = exp(lbx[t-1,u] + S0[t-1]-S0[t] + E_u[t-1]-E_u[t]);  d0_u[0]=0
    d0 = np.zeros((NR + 1, T), np.float32)
    uu = np.arange(1, nrows + 1)
    ld = lbx[:-1, uu] + (S0[:-1] - S0[1:])[:, None] + E[:-1, uu] - E[1:, uu]
    d0[1 : nrows + 1, 1:] = np.exp(ld).T.astype(np.float32)
    v0 = np.exp(-g).astype(np.float32).reshape(C, L)
    coef = np.zeros((C, S * L), np.float32)
    for s in range(1, S + 1):
        for c in range(C):
            u = s - 2 * c
            if 1 <= u <= NR:
                coef[c, (s - 1) * L : s * L] = d0[u, c * L : (c + 1) * L]
    return coef, v0, S0, E[:, nrows]


def _sim_chain(coef, v0):
    """Numpy simulation of the device schedule (fp32), for validation."""
    bufs = [v0.astype(np.float32).copy(), np.zeros((C, L), np.float32)]
    carr = [np.zeros(C, np.float32), np.zeros(C, np.float32)]
    for s in range(1, S + 1):
        cur = bufs[(s - 1) % 2]
        cf = coef[:, (s - 1) * L : s * L]
        state = carr[s % 2].copy()
        out = np.empty((C, L), np.float32)
        for j in range(L):
            state = cf[:, j] * state + cur[:, j]
            out[:, j] = state
        bufs[s % 2][:] = out
        if s <= S - 2:
            carr[s % 2][1:] = out[0 : C - 1, L - 1]
            carr[s % 2][0] = 0.0
    return bufs[S % 2].reshape(-1)


_RUN_STATE = {}


def _prep(inputs):
    lp = np.asarray(inputs["log_probs"], dtype=np.float32)
    tgt = np.asarray(inputs["targets"]).astype(np.int64)
    blank = int(inputs["blank"])
    lb = lp[:, :, :, blank].astype(np.float64)                     # [B,T,U]
    le = np.take_along_axis(
        lp[:, :, : U - 1, :], tgt[:, None, :, None], axis=3
    )[..., 0].astype(np.float64)                                   # [B,T,U-1]

    in_maps, recon = [], []
    sh = _shift_matrix()
    s_ = np.arange(T - 1)
    tau = np.arange(T)

    import ml_dtypes
    bf16 = ml_dtypes.bfloat16

    HW_ = SHEAD * L

    def add_map(coef, v0):
        ini = np.concatenate(
            [v0.astype(bf16), coef[:, :HW_].astype(bf16)], axis=1
        )
        m = {"ini": np.ascontiguousarray(ini), "sh": sh,
             "copt": np.ascontiguousarray(coef[:, HW_:]).astype(bf16)}
        in_maps.append(m)

    for b in range(B):
        coef, v0, S0, Ef = _pack_chain(lb[b, :, : NR + 1], le[b, :, :NR], NR)
        add_map(coef, v0)
        recon.append((S0, Ef))
    for b in range(B):
        lbr = np.zeros((T, NR + 1))
        for v in range(NR + 1):
            lbr[:-1, v] = lb[b, T - 2 - s_, U - 1 - v]
        ler = np.zeros((T, NR))
        for w in range(NR - 1):
            ler[:, w] = le[b, T - 1 - tau, U - 2 - w]
        coef, v0, S0, Er = _pack_chain(lbr, ler, NR - 1)
        add_map(coef, v0)
        recon.append((S0, Er))
    return lb, le, in_maps, recon


def _combine(lb, le, recon, Wf_all):
    tau = np.arange(T)
    costs = np.empty(B, np.float32)
    for b in range(B):
        Wf = Wf_all[b].astype(np.float64)
        Wr = Wf_all[4 + b].astype(np.float64)
        S0f, Ef = recon[b]
        S0r, Er = recon[4 + b]
        with np.errstate(divide="ignore"):
            alphaf = np.log(Wf) + S0f + Ef                          # alpha[t, 48]
            base = lb[b, T - 1, U - 1]
            betar = np.log(Wr) + base + S0r + Er                    # beta-hat[tau, 47]
        beta49 = betar[T - 1 - tau]                                  # beta[t, 49]
        terms = alphaf + le[b, :, NR] + beta49
        mx = terms.max()
        costs[b] = np.float32(-(mx + np.log(np.sum(np.exp(terms - mx)))))
    return costs


def kernel(**inputs) -> np.ndarray:
    _install_shims()
    from concourse.bass_utils import run_bass_kernel_spmd

    lb, le, in_maps, recon = _prep(inputs)
    nc = _build_nc()
    r = run_bass_kernel_spmd(
        nc, in_maps, list(range(8)), trace=_RUN_STATE.get("trace", False)
    )
    _RUN_STATE["last"] = r
    Wf_all = [r.results[i]["outW"].astype(np.float32).reshape(-1) for i in range(8)]
    return _combine(lb, le, recon, Wf_all)
